# revision 4
# baseline (speedup 1.0000x reference)
"""Trainium2 Bass kernel for the BottleneckIndependent MoE-routed conv block.

Math (per sample b):
  rw1 = sigmoid(mean_hw(x) @ r1_w + r1_b)                     [E]
  cw1 = sum_e rw1[e] * w1[e]          (per-sample 1x1 weights)
  out1 = relu(bn1(cw1 @ x))
  rw2 / cw2 / out2: same with 3x3 conv (pad 1)
  rw3 / cw3: 1x1; out = relu(bn3(cw3 @ out2) + x)

Strategy (8 cores, data-parallel over batch, 4 samples/core):
  * BN scales are folded into the expert weights on the host; BN bias + ReLU
    fuse into one ScalarE activation per output chunk.
  * The rank-8 expert combine runs on the PE with the expert weights as the
    STATIONARY operand ([128,128] chunks, rows = (j,e) with j an o-subgroup
    index) against a small block-diagonal routing matrix bd[128, 64]
    (cols = (b,j)).  This yields combined weights directly in
    [i_partition, (b, o)] layout -- exactly the lhsT layout the conv matmuls
    need.  bd is built without any cross-partition ops by pre-replicating the
    routing weights on the host (col m holds expert m%8) and masking.
  * Convs are per-sample matmuls (contraction = input channels); the 3x3 conv
    is 9 shifted 1x1 matmuls accumulating in PSUM over a zero-padded 16x16
    spatial buffer.  The residual add is an identity matmul accumulated into
    the conv3 PSUM group.
  * Everything on device is bf16 except PSUM accumulation / BN bias / pooling
    / routing, which stay fp32.
"""

import numpy as np
import ml_dtypes

B, INP, WIDTH, OUTP, E, H = 32, 1024, 256, 1024, 8, 14
EPS = 1e-5
S = H * H            # 196
SP = 256             # 16*16 padded spatial
NCORES = 8
BS = B // NCORES     # 4 samples per core
P = 128

BF16 = ml_dtypes.bfloat16

# stage geometry: (n_ichunks, n_ogroups, taps)
#   stage1: i=1024 (8 chunks), o=256 (16 groups of 16), 1 tap
#   stage2: i=256 (2 chunks),  o=256 (16 groups), 9 taps
#   stage3: i=256 (2 chunks),  o=1024 (64 groups), 1 tap
N_CHUNKS1 = 8 * 16            # (ic, g)
N_CHUNKS2 = 9 * 2 * 16        # (tap, ic, g)
N_CHUNKS3 = 2 * 64            # (ic, g)

_nc_cache = None
last_exec_time_ns = None
last_trace_path = None


# ----------------------------------------------------------------------------
# Host-side input preparation (pure numpy)
# ----------------------------------------------------------------------------

def _fold_bn(g, b, m, v):
    inv = (g / np.sqrt(v + EPS)).astype(np.float32)
    beta = (b - m * inv).astype(np.float32)
    return inv, beta


def _prep_weights(w1, w2, w3, r1_w, r1_b, r2_w, r2_b, r3_w, r3_b,
                  bn1_g, bn1_b, bn1_m, bn1_v, bn2_g, bn2_b, bn2_m, bn2_v,
                  bn3_g, bn3_b, bn3_m, bn3_v):
    inv1, beta1 = _fold_bn(bn1_g, bn1_b, bn1_m, bn1_v)
    inv2, beta2 = _fold_bn(bn2_g, bn2_b, bn2_m, bn2_v)
    inv3, beta3 = _fold_bn(bn3_g, bn3_b, bn3_m, bn3_v)

    w1p = (w1[:, :, :, 0, 0] * inv1[None, :, None]).astype(np.float32)  # [E,256,1024]
    w2p = (w2 * inv2[None, :, None, None, None]).astype(np.float32)     # [E,256,256,3,3]
    w3p = (w3[:, :, :, 0, 0] * inv3[None, :, None]).astype(np.float32)  # [E,1024,256]

    # stage1: rows (j,e); cols (ic, g, ip); value = w1p[e, g*16+j, ic*128+ip]
    a = w1p.reshape(E, 16, 16, 8, P)              # e, g, j, ic, ip
    w1r = a.transpose(2, 0, 3, 1, 4).reshape(P, 8 * 16 * P).astype(BF16)

    # stage2: cols (tap=kh*3+kw, ic, g, ip); value = w2p[e, g*16+j, ic*128+ip, kh, kw]
    a = w2p.reshape(E, 16, 16, 2, P, 3, 3)        # e, g, j, ic, ip, kh, kw
    w2r = a.transpose(2, 0, 5, 6, 3, 1, 4).reshape(P, 9 * 2 * 16 * P).astype(BF16)

    # stage3: cols (ic, g(64), ip); value = w3p[e, g*16+j, ic*128+ip]
    a = w3p.reshape(E, 64, 16, 2, P)              # e, g, j, ic, ip
    w3r = a.transpose(2, 0, 3, 1, 4).reshape(P, 2 * 64 * P).astype(BF16)

    def rep_routing(rw, nchunks):
        # [C, E] -> [128, nchunks*128] fp32; col m of chunk ic = rw[ic*128+p, m%8]/S
        r = (rw / float(S)).astype(np.float32).reshape(nchunks, P, E)
        rrep = np.tile(r[:, :, None, :], (1, 1, 16, 1)).reshape(nchunks, P, P)
        return rrep.transpose(1, 0, 2).reshape(P, nchunks * P)

    r1rep = rep_routing(r1_w, 8)
    r2rep = rep_routing(r2_w, 2)
    r3rep = rep_routing(r3_w, 2)

    rb_rep = np.stack(
        [np.tile(np.asarray(rb, np.float32), 16) for rb in (r1_b, r2_b, r3_b)], axis=1
    )  # [128, 3]

    # mask[p, b*16+j] = 1 if j == p//8
    jj = np.arange(P)[:, None] // 8                      # [128,1]
    col_j = np.tile(np.arange(16), 4)[None, :]           # [1,64] (b-major)
    mask = (col_j == jj).astype(BF16)                    # [128,64]

    beta = np.concatenate(
        [beta1.reshape(2, P).T, beta2.reshape(2, P).T, beta3.reshape(8, P).T], axis=1
    ).astype(np.float32)                                 # [128, 12]

    ident = np.eye(P, dtype=BF16)

    return dict(w1r=w1r, w2r=w2r, w3r=w3r, r1rep=r1rep, r2rep=r2rep, r3rep=r3rep,
                rb_rep=rb_rep, mask=mask, beta=beta, ident=ident)


def _prep_x(x):
    # x [B, 1024, 14, 14] -> per-core [128, BS*8*196] bf16,
    # col = b*1568 + ic*196 + s, partition = i % 128 (i = ic*128+p)
    out = []
    for c in range(NCORES):
        xc = np.asarray(x[c * BS:(c + 1) * BS], np.float32)
        xb = xc.reshape(BS, 8, P, S).transpose(2, 0, 1, 3).reshape(P, BS * 8 * S)
        out.append(np.ascontiguousarray(xb.astype(BF16)))
    return out


# ----------------------------------------------------------------------------
# Device program
# ----------------------------------------------------------------------------

def _build_nc():
    import concourse.tile as tile
    import concourse.mybir as mybir
    from concourse.bacc import Bacc
    from contextlib import ExitStack

    f32 = mybir.dt.float32
    bf16 = mybir.dt.bfloat16
    AF = mybir.ActivationFunctionType
    ALU = mybir.AluOpType
    AX = mybir.AxisListType

    nc = Bacc("TRN2")

    xd = nc.dram_tensor("x_bf", [P, BS * 8 * S], bf16, kind="ExternalInput")
    w1d = nc.dram_tensor("w1r", [P, N_CHUNKS1 * P], bf16, kind="ExternalInput")
    w2d = nc.dram_tensor("w2r", [P, N_CHUNKS2 * P], bf16, kind="ExternalInput")
    w3d = nc.dram_tensor("w3r", [P, N_CHUNKS3 * P], bf16, kind="ExternalInput")
    r1d = nc.dram_tensor("r1rep", [P, 8 * P], f32, kind="ExternalInput")
    r2d = nc.dram_tensor("r2rep", [P, 2 * P], f32, kind="ExternalInput")
    r3d = nc.dram_tensor("r3rep", [P, 2 * P], f32, kind="ExternalInput")
    rbd = nc.dram_tensor("rb_rep", [P, 3], f32, kind="ExternalInput")
    maskd = nc.dram_tensor("mask", [P, 64], bf16, kind="ExternalInput")
    betad = nc.dram_tensor("beta", [P, 12], f32, kind="ExternalInput")
    identd = nc.dram_tensor("ident", [P, P], bf16, kind="ExternalInput")
    outd = nc.dram_tensor("out", [P, BS * 8 * S], f32, kind="ExternalOutput")

    with tile.TileContext(nc) as tc, ExitStack() as ctx:
        singles = ctx.enter_context(tc.tile_pool(name="singles", bufs=1))
        wbig = ctx.enter_context(tc.tile_pool(name="wbig", bufs=1))
        cwa = ctx.enter_context(tc.tile_pool(name="cwa", bufs=1))
        ostage = ctx.enter_context(tc.tile_pool(name="ostage", bufs=2))
        kpsum = ctx.enter_context(tc.tile_pool(name="kpsum", bufs=3, space="PSUM"))
        cpsum = ctx.enter_context(tc.tile_pool(name="cpsum", bufs=3, space="PSUM"))
        rpsum = ctx.enter_context(tc.tile_pool(name="rpsum", bufs=2, space="PSUM"))

        # ---- constants / small tensors -----------------------------------
        mask_sb = singles.tile([P, 64], bf16)
        nc.sync.dma_start(out=mask_sb, in_=maskd[:, :])
        ident_sb = singles.tile([P, P], bf16)
        nc.sync.dma_start(out=ident_sb, in_=identd[:, :])
        rb_sb = singles.tile([P, 3], f32)
        nc.sync.dma_start(out=rb_sb, in_=rbd[:, :])
        beta_sb = singles.tile([P, 12], f32)
        nc.sync.dma_start(out=beta_sb, in_=betad[:, :])
        r1w_sb = singles.tile([P, 8 * P], f32)
        nc.sync.dma_start(out=r1w_sb, in_=r1d[:, :])
        r2w_sb = singles.tile([P, 2 * P], f32)
        nc.sync.dma_start(out=r2w_sb, in_=r2d[:, :])
        r3w_sb = singles.tile([P, 2 * P], f32)
        nc.sync.dma_start(out=r3w_sb, in_=r3d[:, :])

        # ---- big SBUF tensors --------------------------------------------
        x_sb = singles.tile([P, BS * 8 * S], bf16)
        for b in range(BS):
            nc.sync.dma_start(out=x_sb[:, b * 8 * S:(b + 1) * 8 * S],
                              in_=xd[:, b * 8 * S:(b + 1) * 8 * S])

        w1_sb = wbig.tile([P, N_CHUNKS1 * P], bf16, tag="wbig", name="w1_sb")
        for sl in range(8):
            w = N_CHUNKS1 * P // 8
            nc.sync.dma_start(out=w1_sb[:, sl * w:(sl + 1) * w],
                              in_=w1d[:, sl * w:(sl + 1) * w])
        w2_sb = singles.tile([P, N_CHUNKS2 * P], bf16)
        for sl in range(16):
            w = N_CHUNKS2 * P // 16
            nc.sync.dma_start(out=w2_sb[:, sl * w:(sl + 1) * w],
                              in_=w2d[:, sl * w:(sl + 1) * w])

        cw1 = cwa.tile([P, BS * 8 * 256], bf16, tag="cwa", name="cw1")
        cw2 = singles.tile([P, BS * 9 * 2 * 256], bf16)
        out1pad = singles.tile([P, BS * 2 * SP], bf16)
        nc.vector.memset(out1pad, 0.0)
        out2 = singles.tile([P, BS * 2 * S], bf16)

        pool1 = singles.tile([P, 8 * BS], f32)
        pool2 = singles.tile([P, 2 * BS], f32)
        pool3 = singles.tile([P, 2 * BS], f32)

        x_v = x_sb.rearrange("p (b c s) -> p b c s", b=BS, c=8)
        mask_v = mask_sb.rearrange("p (b j) -> p b j", b=BS)
        out1pad_v = out1pad.rearrange("p (b c h w) -> p b c h w", b=BS, c=2, h=16)

        # ---- pooling 1 (mean over spatial; 1/S folded into routing w) ----
        pool1_v = pool1.rearrange("p (c b) -> p c b", b=BS)
        for b in range(BS):
            nc.vector.tensor_reduce(out=pool1_v[:, :, b], in_=x_v[:, b],
                                    axis=AX.X, op=ALU.add)

        # ---- routing helper ----------------------------------------------
        def routing(st, rw_sb, pool_sb, nchunks):
            ps = rpsum.tile([P, BS], f32, tag="rps", name=f"ps_rt{st}")
            for ic in range(nchunks):
                nc.tensor.matmul(ps, rw_sb[:, ic * P:(ic + 1) * P],
                                 pool_sb[:, ic * BS:(ic + 1) * BS],
                                 start=(ic == 0), stop=(ic == nchunks - 1))
            rwt = singles.tile([P, BS], bf16, name=f"rwt{st}")
            nc.scalar.activation(out=rwt, in_=ps, func=AF.Sigmoid,
                                 bias=rb_sb[:, st:st + 1], scale=1.0)
            bd = singles.tile([P, BS * 16], bf16, name=f"bd{st}")
            bd_v = bd.rearrange("p (b j) -> p b j", b=BS)
            nc.vector.tensor_tensor(
                out=bd_v, in0=mask_v,
                in1=rwt[:, :, None].to_broadcast((P, BS, 16)),
                op=ALU.mult)
            return bd

        # ---- combine helper ----------------------------------------------
        def combine(st, w_sb, bd, nbanks, dst_fn):
            for bank in range(nbanks):
                ps = kpsum.tile([P, 512], f32, tag="kps", name=f"ps_cmb{st}")
                for c8 in range(8):
                    c = bank * 8 + c8
                    nc.tensor.matmul(ps[:, c8 * 64:(c8 + 1) * 64],
                                     w_sb[:, c * P:(c + 1) * P], bd,
                                     start=True, stop=True)
                src = ps.rearrange("p (c b j) -> p c b j", c=8, b=BS)
                dst = dst_fn(bank)
                if bank % 2 == 0:
                    nc.vector.tensor_copy(out=dst, in_=src)
                else:
                    nc.scalar.copy(dst, src)

        # ================== stage 1 =======================================
        bd1 = routing(0, r1w_sb, pool1, 8)
        cw1_v = cw1.rearrange("p (b ic g j) -> p ic g b j", b=BS, ic=8, g=16, j=16)
        combine(1, w1_sb, bd1, N_CHUNKS1 // 8,
                lambda bank: cw1_v[:, bank // 2, (bank % 2) * 8:(bank % 2) * 8 + 8])

        # w3 DMA after combine1 frees the wbig slot
        w3_sb = wbig.tile([P, N_CHUNKS3 * P], bf16, tag="wbig", name="w3_sb")
        for sl in range(8):
            w = N_CHUNKS3 * P // 8
            nc.sync.dma_start(out=w3_sb[:, sl * w:(sl + 1) * w],
                              in_=w3d[:, sl * w:(sl + 1) * w])

        # conv1 + bn1 + relu + pool2
        for b in range(BS):
            for oc in range(2):
                ps = cpsum.tile([P, S], f32, tag="cps", name="ps_c1")
                for ic in range(8):
                    nc.tensor.matmul(
                        ps, cw1[:, b * 2048 + ic * 256 + oc * P:
                                b * 2048 + ic * 256 + oc * P + P],
                        x_sb[:, b * 8 * S + ic * S:b * 8 * S + (ic + 1) * S],
                        start=(ic == 0), stop=(ic == 7))
                nc.scalar.activation(
                    out=out1pad_v[:, b, oc, 1:15, 1:15],
                    in_=ps.rearrange("p (h w) -> p h w", h=H),
                    func=AF.Relu, bias=beta_sb[:, oc:oc + 1], scale=1.0)
            for oc in range(2):
                nc.vector.tensor_reduce(
                    out=pool2[:, oc * BS + b:oc * BS + b + 1],
                    in_=out1pad[:, (b * 2 + oc) * SP:(b * 2 + oc + 1) * SP],
                    axis=AX.X, op=ALU.add)

        # ================== stage 2 =======================================
        bd2 = routing(1, r2w_sb, pool2, 2)
        cw2_v = cw2.rearrange("p (b t ic g j) -> p t ic g b j", b=BS, t=9, ic=2, g=16, j=16)
        combine(2, w2_sb, bd2, N_CHUNKS2 // 8,
                lambda bank: cw2_v[:, bank // 4, (bank % 4) // 2,
                                   (bank % 2) * 8:(bank % 2) * 8 + 8])

        # conv2 + bn2 + relu + pool3
        for b in range(BS):
            for oc in range(2):
                ps = cpsum.tile([P, S], f32, tag="cps", name="ps_c2")
                k = 0
                for ic in range(2):
                    for tap in range(9):
                        kh, kw = tap // 3, tap % 3
                        nc.tensor.matmul(
                            ps.rearrange("p (h w) -> p h w", h=H),
                            cw2[:, b * 4608 + tap * 512 + ic * 256 + oc * P:
                                b * 4608 + tap * 512 + ic * 256 + oc * P + P],
                            out1pad_v[:, b, ic, kh:kh + H, kw:kw + H],
                            start=(k == 0), stop=(k == 17))
                        k += 1
                nc.scalar.activation(
                    out=out2[:, (b * 2 + oc) * S:(b * 2 + oc + 1) * S],
                    in_=ps, func=AF.Relu, bias=beta_sb[:, 2 + oc:3 + oc], scale=1.0)
            for oc in range(2):
                nc.vector.tensor_reduce(
                    out=pool3[:, oc * BS + b:oc * BS + b + 1],
                    in_=out2[:, (b * 2 + oc) * S:(b * 2 + oc + 1) * S],
                    axis=AX.X, op=ALU.add)

        # ================== stage 3 =======================================
        bd3 = routing(2, r3w_sb, pool3, 2)
        cw3 = cwa.tile([P, BS * 2 * 1024], bf16, tag="cwa", name="cw3")
        cw3_v = cw3.rearrange("p (b ic g j) -> p ic g b j", b=BS, ic=2, g=64, j=16)
        combine(3, w3_sb, bd3, N_CHUNKS3 // 8,
                lambda bank: cw3_v[:, bank // 8, (bank % 8) * 8:(bank % 8) * 8 + 8])

        # conv3 + identity + bn3-bias + relu -> out
        for b in range(BS):
            ost = ostage.tile([P, 8 * S], f32, tag="ost", name="ost")
            for oc in range(8):
                ps = cpsum.tile([P, S], f32, tag="cps", name="ps_c3")
                nc.tensor.matmul(
                    ps, cw3[:, b * 2048 + 0 * 1024 + oc * P:
                            b * 2048 + 0 * 1024 + oc * P + P],
                    out2[:, b * 2 * S:b * 2 * S + S], start=True, stop=False)
                nc.tensor.matmul(
                    ps, cw3[:, b * 2048 + 1 * 1024 + oc * P:
                            b * 2048 + 1 * 1024 + oc * P + P],
                    out2[:, b * 2 * S + S:b * 2 * S + 2 * S], start=False, stop=False)
                nc.tensor.matmul(
                    ps, ident_sb,
                    x_sb[:, b * 8 * S + oc * S:b * 8 * S + (oc + 1) * S],
                    start=False, stop=True)
                nc.scalar.activation(
                    out=ost[:, oc * S:(oc + 1) * S], in_=ps, func=AF.Relu,
                    bias=beta_sb[:, 4 + oc:5 + oc], scale=1.0)
            nc.sync.dma_start(out=outd[:, b * 8 * S:(b + 1) * 8 * S], in_=ost)

    nc.finalize()
    return nc


# ----------------------------------------------------------------------------
# Entry point
# ----------------------------------------------------------------------------

def kernel(x, w1, w2, w3, r1_w, r1_b, r2_w, r2_b, r3_w, r3_b,
           bn1_g, bn1_b, bn1_m, bn1_v, bn2_g, bn2_b, bn2_m, bn2_v,
           bn3_g, bn3_b, bn3_m, bn3_v, _trace=False):
    global _nc_cache, last_exec_time_ns, last_trace_path
    from concourse.bass_utils import run_bass_kernel_spmd

    shared = _prep_weights(
        np.asarray(w1, np.float32), np.asarray(w2, np.float32),
        np.asarray(w3, np.float32),
        np.asarray(r1_w, np.float32), np.asarray(r1_b, np.float32),
        np.asarray(r2_w, np.float32), np.asarray(r2_b, np.float32),
        np.asarray(r3_w, np.float32), np.asarray(r3_b, np.float32),
        np.asarray(bn1_g, np.float32), np.asarray(bn1_b, np.float32),
        np.asarray(bn1_m, np.float32), np.asarray(bn1_v, np.float32),
        np.asarray(bn2_g, np.float32), np.asarray(bn2_b, np.float32),
        np.asarray(bn2_m, np.float32), np.asarray(bn2_v, np.float32),
        np.asarray(bn3_g, np.float32), np.asarray(bn3_b, np.float32),
        np.asarray(bn3_m, np.float32), np.asarray(bn3_v, np.float32))
    xs = _prep_x(np.asarray(x, np.float32))

    shared_map = {
        "w1r": shared["w1r"], "w2r": shared["w2r"], "w3r": shared["w3r"],
        "r1rep": shared["r1rep"].astype(np.float32),
        "r2rep": shared["r2rep"].astype(np.float32),
        "r3rep": shared["r3rep"].astype(np.float32),
        "rb_rep": shared["rb_rep"].astype(np.float32),
        "mask": shared["mask"], "beta": shared["beta"].astype(np.float32),
        "ident": shared["ident"],
    }
    in_maps = [dict(shared_map, x_bf=xs[c]) for c in range(NCORES)]

    if _nc_cache is None:
        _nc_cache = _build_nc()
    res = run_bass_kernel_spmd(_nc_cache, in_maps, core_ids=list(range(NCORES)),
                               trace=_trace)
    last_exec_time_ns = res.exec_time_ns
    last_trace_path = (res.instructions_and_trace or (None, None))[1]

    out = np.empty((B, OUTP, H, H), np.float32)
    for c in range(NCORES):
        o = res.results[c]["out"]                       # [128, BS*8*196] f32
        out[c * BS:(c + 1) * BS] = (
            o.reshape(P, BS, 8, S).transpose(1, 2, 0, 3).reshape(BS, OUTP, H, H))
    return out


# revision 5
# speedup vs baseline: 1.0174x; 1.0174x over previous
"""Trainium2 Bass kernel for the BottleneckIndependent MoE-routed conv block.

Math (per sample b):
  rw1 = sigmoid(mean_hw(x) @ r1_w + r1_b)                     [E]
  cw1 = sum_e rw1[e] * w1[e]          (per-sample 1x1 weights)
  out1 = relu(bn1(cw1 @ x))
  rw2 / cw2 / out2: same with 3x3 conv (pad 1)
  rw3 / cw3: 1x1; out = relu(bn3(cw3 @ out2) + x)

Strategy (8 cores, data-parallel over batch, 4 samples/core):
  * BN scales are folded into the expert weights on the host; BN bias + ReLU
    fuse into one ScalarE activation per output chunk.
  * The rank-8 expert combine runs on the PE with the expert weights as the
    STATIONARY operand ([128,128] chunks, rows = (j,e) with j an o-subgroup
    index) against a small block-diagonal routing matrix bd[128, 64]
    (cols = (b,j)).  This yields combined weights directly in
    [i_partition, (b, o)] layout -- exactly the lhsT layout the conv matmuls
    need.  bd is built without any cross-partition ops by pre-replicating the
    routing weights on the host (col m holds expert m%8) and masking.
  * Convs are per-sample matmuls (contraction = input channels); the 3x3 conv
    is 9 shifted 1x1 matmuls accumulating in PSUM over a zero-padded 16x16
    spatial buffer.  The residual add is an identity matmul accumulated into
    the conv3 PSUM group.
  * Everything on device is bf16 except PSUM accumulation / BN bias / pooling
    / routing, which stay fp32.
"""

import numpy as np
import ml_dtypes

B, INP, WIDTH, OUTP, E, H = 32, 1024, 256, 1024, 8, 14
EPS = 1e-5
S = H * H            # 196
SP = 256             # 16*16 padded spatial
NCORES = 8
BS = B // NCORES     # 4 samples per core
P = 128

BF16 = ml_dtypes.bfloat16

# stage geometry: (n_ichunks, n_ogroups, taps)
#   stage1: i=1024 (8 chunks), o=256 (16 groups of 16), 1 tap
#   stage2: i=256 (2 chunks),  o=256 (16 groups), 9 taps
#   stage3: i=256 (2 chunks),  o=1024 (64 groups), 1 tap
N_CHUNKS1 = 8 * 16            # (ic, g)
N_CHUNKS2 = 9 * 2 * 16        # (tap, ic, g)
N_CHUNKS3 = 2 * 64            # (ic, g)

_nc_cache = None
last_exec_time_ns = None
last_trace_path = None


# ----------------------------------------------------------------------------
# Host-side input preparation (pure numpy)
# ----------------------------------------------------------------------------

def _fold_bn(g, b, m, v):
    inv = (g / np.sqrt(v + EPS)).astype(np.float32)
    beta = (b - m * inv).astype(np.float32)
    return inv, beta


def _prep_weights(w1, w2, w3, r1_w, r1_b, r2_w, r2_b, r3_w, r3_b,
                  bn1_g, bn1_b, bn1_m, bn1_v, bn2_g, bn2_b, bn2_m, bn2_v,
                  bn3_g, bn3_b, bn3_m, bn3_v):
    inv1, beta1 = _fold_bn(bn1_g, bn1_b, bn1_m, bn1_v)
    inv2, beta2 = _fold_bn(bn2_g, bn2_b, bn2_m, bn2_v)
    inv3, beta3 = _fold_bn(bn3_g, bn3_b, bn3_m, bn3_v)

    w1p = (w1[:, :, :, 0, 0] * inv1[None, :, None]).astype(np.float32)  # [E,256,1024]
    w2p = (w2 * inv2[None, :, None, None, None]).astype(np.float32)     # [E,256,256,3,3]
    w3p = (w3[:, :, :, 0, 0] * inv3[None, :, None]).astype(np.float32)  # [E,1024,256]

    # stage1: rows (j,e); cols (ic, g, ip); value = w1p[e, g*16+j, ic*128+ip]
    a = w1p.reshape(E, 16, 16, 8, P)              # e, g, j, ic, ip
    w1r = a.transpose(2, 0, 3, 1, 4).reshape(P, 8 * 16 * P).astype(BF16)

    # stage2: cols (tap=kh*3+kw, ic, g, ip); value = w2p[e, g*16+j, ic*128+ip, kh, kw]
    a = w2p.reshape(E, 16, 16, 2, P, 3, 3)        # e, g, j, ic, ip, kh, kw
    w2r = a.transpose(2, 0, 5, 6, 3, 1, 4).reshape(P, 9 * 2 * 16 * P).astype(BF16)

    # stage3: cols (ic, g(64), ip); value = w3p[e, g*16+j, ic*128+ip]
    a = w3p.reshape(E, 64, 16, 2, P)              # e, g, j, ic, ip
    w3r = a.transpose(2, 0, 3, 1, 4).reshape(P, 2 * 64 * P).astype(BF16)

    def rep_routing(rw, nchunks):
        # [C, E] -> [128, nchunks*128] fp32; col m of chunk ic = rw[ic*128+p, m%8]/S
        r = (rw / float(S)).astype(np.float32).reshape(nchunks, P, E)
        rrep = np.tile(r[:, :, None, :], (1, 1, 16, 1)).reshape(nchunks, P, P)
        return rrep.transpose(1, 0, 2).reshape(P, nchunks * P)

    r1rep = rep_routing(r1_w, 8)
    r2rep = rep_routing(r2_w, 2)
    r3rep = rep_routing(r3_w, 2)

    rb_rep = np.stack(
        [np.tile(np.asarray(rb, np.float32), 16) for rb in (r1_b, r2_b, r3_b)], axis=1
    )  # [128, 3]

    # mask[p, b*16+j] = 1 if j == p//8
    jj = np.arange(P)[:, None] // 8                      # [128,1]
    col_j = np.tile(np.arange(16), 4)[None, :]           # [1,64] (b-major)
    mask = (col_j == jj).astype(BF16)                    # [128,64]

    beta = np.concatenate(
        [beta1.reshape(2, P).T, beta2.reshape(2, P).T, beta3.reshape(8, P).T], axis=1
    ).astype(np.float32)                                 # [128, 12]

    ident = np.eye(P, dtype=BF16)

    return dict(w1r=w1r, w2r=w2r, w3r=w3r, r1rep=r1rep, r2rep=r2rep, r3rep=r3rep,
                rb_rep=rb_rep, mask=mask, beta=beta, ident=ident)


def _prep_x(x):
    # x [B, 1024, 14, 14] -> per-core [128, BS*8*196] bf16,
    # col = b*1568 + ic*196 + s, partition = i % 128 (i = ic*128+p)
    out = []
    for c in range(NCORES):
        xc = np.asarray(x[c * BS:(c + 1) * BS], np.float32)
        xb = xc.reshape(BS, 8, P, S).transpose(2, 0, 1, 3).reshape(P, BS * 8 * S)
        out.append(np.ascontiguousarray(xb.astype(BF16)))
    return out


# ----------------------------------------------------------------------------
# Device program
# ----------------------------------------------------------------------------

def _build_nc():
    import concourse.tile as tile
    import concourse.mybir as mybir
    from concourse.bacc import Bacc
    from contextlib import ExitStack

    f32 = mybir.dt.float32
    bf16 = mybir.dt.bfloat16
    AF = mybir.ActivationFunctionType
    ALU = mybir.AluOpType
    AX = mybir.AxisListType

    nc = Bacc("TRN2")

    xd = nc.dram_tensor("x_bf", [P, BS * 8 * S], bf16, kind="ExternalInput")
    w1d = nc.dram_tensor("w1r", [P, N_CHUNKS1 * P], bf16, kind="ExternalInput")
    w2d = nc.dram_tensor("w2r", [P, N_CHUNKS2 * P], bf16, kind="ExternalInput")
    w3d = nc.dram_tensor("w3r", [P, N_CHUNKS3 * P], bf16, kind="ExternalInput")
    r1d = nc.dram_tensor("r1rep", [P, 8 * P], f32, kind="ExternalInput")
    r2d = nc.dram_tensor("r2rep", [P, 2 * P], f32, kind="ExternalInput")
    r3d = nc.dram_tensor("r3rep", [P, 2 * P], f32, kind="ExternalInput")
    rbd = nc.dram_tensor("rb_rep", [P, 3], f32, kind="ExternalInput")
    maskd = nc.dram_tensor("mask", [P, 64], bf16, kind="ExternalInput")
    betad = nc.dram_tensor("beta", [P, 12], f32, kind="ExternalInput")
    identd = nc.dram_tensor("ident", [P, P], bf16, kind="ExternalInput")
    outd = nc.dram_tensor("out", [P, BS * 8 * S], f32, kind="ExternalOutput")

    with tile.TileContext(nc) as tc, ExitStack() as ctx:
        singles = ctx.enter_context(tc.tile_pool(name="singles", bufs=1))
        wbig = ctx.enter_context(tc.tile_pool(name="wbig", bufs=1))
        cwa = ctx.enter_context(tc.tile_pool(name="cwa", bufs=1))
        ostage = ctx.enter_context(tc.tile_pool(name="ostage", bufs=2))
        kpsum = ctx.enter_context(tc.tile_pool(name="kpsum", bufs=3, space="PSUM"))
        cpsum = ctx.enter_context(tc.tile_pool(name="cpsum", bufs=3, space="PSUM"))
        rpsum = ctx.enter_context(tc.tile_pool(name="rpsum", bufs=2, space="PSUM"))

        # ---- constants / small tensors -----------------------------------
        mask_sb = singles.tile([P, 64], bf16)
        nc.sync.dma_start(out=mask_sb, in_=maskd[:, :])
        ident_sb = singles.tile([P, P], bf16)
        nc.sync.dma_start(out=ident_sb, in_=identd[:, :])
        rb_sb = singles.tile([P, 3], f32)
        nc.sync.dma_start(out=rb_sb, in_=rbd[:, :])
        beta_sb = singles.tile([P, 12], f32)
        nc.sync.dma_start(out=beta_sb, in_=betad[:, :])
        r1w_sb = singles.tile([P, 8 * P], f32)
        nc.sync.dma_start(out=r1w_sb, in_=r1d[:, :])
        r2w_sb = singles.tile([P, 2 * P], f32)
        nc.sync.dma_start(out=r2w_sb, in_=r2d[:, :])
        r3w_sb = singles.tile([P, 2 * P], f32)
        nc.sync.dma_start(out=r3w_sb, in_=r3d[:, :])

        # ---- big SBUF tensors --------------------------------------------
        x_sb = singles.tile([P, BS * 8 * S], bf16)
        for b in range(BS):
            nc.sync.dma_start(out=x_sb[:, b * 8 * S:(b + 1) * 8 * S],
                              in_=xd[:, b * 8 * S:(b + 1) * 8 * S])

        w1_sb = wbig.tile([P, N_CHUNKS1 * P], bf16, tag="wbig", name="w1_sb")
        for sl in range(8):
            w = N_CHUNKS1 * P // 8
            nc.sync.dma_start(out=w1_sb[:, sl * w:(sl + 1) * w],
                              in_=w1d[:, sl * w:(sl + 1) * w])
        w2_sb = singles.tile([P, N_CHUNKS2 * P], bf16)
        for sl in range(16):
            w = N_CHUNKS2 * P // 16
            nc.sync.dma_start(out=w2_sb[:, sl * w:(sl + 1) * w],
                              in_=w2d[:, sl * w:(sl + 1) * w])

        cw1 = cwa.tile([P, BS * 8 * 256], bf16, tag="cwa", name="cw1")
        cw2 = singles.tile([P, BS * 9 * 2 * 256], bf16)
        out1pad = singles.tile([P, BS * 2 * SP], bf16)
        nc.vector.memset(out1pad, 0.0)
        out2 = singles.tile([P, BS * 2 * S], bf16)

        pool1 = singles.tile([P, 8 * BS], f32)
        pool2 = singles.tile([P, 2 * BS], f32)
        pool3 = singles.tile([P, 2 * BS], f32)

        x_v = x_sb.rearrange("p (b c s) -> p b c s", b=BS, c=8)
        mask_v = mask_sb.rearrange("p (b j) -> p b j", b=BS)
        out1pad_v = out1pad.rearrange("p (b c h w) -> p b c h w", b=BS, c=2, h=16)

        # ---- pooling 1 (mean over spatial; 1/S folded into routing w) ----
        pool1_v = pool1.rearrange("p (c b) -> p c b", b=BS)
        for b in range(BS):
            nc.vector.tensor_reduce(out=pool1_v[:, :, b], in_=x_v[:, b],
                                    axis=AX.X, op=ALU.add)

        # ---- routing helper ----------------------------------------------
        def routing(st, rw_sb, pool_sb, nchunks):
            ps = rpsum.tile([P, BS], f32, tag="rps", name=f"ps_rt{st}")
            for ic in range(nchunks):
                nc.tensor.matmul(ps, rw_sb[:, ic * P:(ic + 1) * P],
                                 pool_sb[:, ic * BS:(ic + 1) * BS],
                                 start=(ic == 0), stop=(ic == nchunks - 1))
            rwt = singles.tile([P, BS], bf16, name=f"rwt{st}")
            nc.scalar.activation(out=rwt, in_=ps, func=AF.Sigmoid,
                                 bias=rb_sb[:, st:st + 1], scale=1.0)
            bd = singles.tile([P, BS * 16], bf16, name=f"bd{st}")
            bd_v = bd.rearrange("p (b j) -> p b j", b=BS)
            nc.vector.tensor_tensor(
                out=bd_v, in0=mask_v,
                in1=rwt[:, :, None].to_broadcast((P, BS, 16)),
                op=ALU.mult)
            return bd

        # ---- combine helper ----------------------------------------------
        def combine(st, w_sb, bd, nbanks, dst_fn):
            for bank in range(nbanks):
                ps = kpsum.tile([P, 512], f32, tag="kps", name=f"ps_cmb{st}")
                for c8 in range(8):
                    c = bank * 8 + c8
                    nc.tensor.matmul(ps[:, c8 * 64:(c8 + 1) * 64],
                                     w_sb[:, c * P:(c + 1) * P], bd,
                                     start=True, stop=True)
                src = ps.rearrange("p (c b j) -> p c b j", c=8, b=BS)
                dst = dst_fn(bank)
                if bank % 2 == 0:
                    nc.vector.tensor_copy(out=dst, in_=src)
                else:
                    nc.scalar.copy(dst, src)

        # ================== stage 1 =======================================
        bd1 = routing(0, r1w_sb, pool1, 8)
        cw1_v = cw1.rearrange("p (b ic g j) -> p ic g b j", b=BS, ic=8, g=16, j=16)
        combine(1, w1_sb, bd1, N_CHUNKS1 // 8,
                lambda bank: cw1_v[:, bank // 2, (bank % 2) * 8:(bank % 2) * 8 + 8])

        # w3 DMA after combine1 frees the wbig slot
        w3_sb = wbig.tile([P, N_CHUNKS3 * P], bf16, tag="wbig", name="w3_sb")
        for sl in range(8):
            w = N_CHUNKS3 * P // 8
            nc.sync.dma_start(out=w3_sb[:, sl * w:(sl + 1) * w],
                              in_=w3d[:, sl * w:(sl + 1) * w])

        # conv1 + bn1 + relu + pool2
        for b in range(BS):
            for oc in range(2):
                ps = cpsum.tile([P, S], f32, tag="cps", name="ps_c1")
                for ic in range(8):
                    nc.tensor.matmul(
                        ps, cw1[:, b * 2048 + ic * 256 + oc * P:
                                b * 2048 + ic * 256 + oc * P + P],
                        x_sb[:, b * 8 * S + ic * S:b * 8 * S + (ic + 1) * S],
                        start=(ic == 0), stop=(ic == 7))
                nc.scalar.activation(
                    out=out1pad_v[:, b, oc, 1:15, 1:15],
                    in_=ps.rearrange("p (h w) -> p h w", h=H),
                    func=AF.Relu, bias=beta_sb[:, oc:oc + 1], scale=1.0)
            for oc in range(2):
                nc.vector.tensor_reduce(
                    out=pool2[:, oc * BS + b:oc * BS + b + 1],
                    in_=out1pad[:, (b * 2 + oc) * SP:(b * 2 + oc + 1) * SP],
                    axis=AX.X, op=ALU.add)

        # ================== stage 2 =======================================
        bd2 = routing(1, r2w_sb, pool2, 2)
        cw2_v = cw2.rearrange("p (b t ic g j) -> p t ic g b j", b=BS, t=9, ic=2, g=16, j=16)
        combine(2, w2_sb, bd2, N_CHUNKS2 // 8,
                lambda bank: cw2_v[:, bank // 4, (bank % 4) // 2,
                                   (bank % 2) * 8:(bank % 2) * 8 + 8])

        # conv2 + bn2 + relu + pool3
        for b in range(BS):
            for oc in range(2):
                ps = cpsum.tile([P, S], f32, tag="cps", name="ps_c2")
                k = 0
                for ic in range(2):
                    for tap in range(9):
                        kh, kw = tap // 3, tap % 3
                        nc.tensor.matmul(
                            ps.rearrange("p (h w) -> p h w", h=H),
                            cw2[:, b * 4608 + tap * 512 + ic * 256 + oc * P:
                                b * 4608 + tap * 512 + ic * 256 + oc * P + P],
                            out1pad_v[:, b, ic, kh:kh + H, kw:kw + H],
                            start=(k == 0), stop=(k == 17))
                        k += 1
                nc.scalar.activation(
                    out=out2[:, (b * 2 + oc) * S:(b * 2 + oc + 1) * S],
                    in_=ps, func=AF.Relu, bias=beta_sb[:, 2 + oc:3 + oc], scale=1.0)
            for oc in range(2):
                nc.vector.tensor_reduce(
                    out=pool3[:, oc * BS + b:oc * BS + b + 1],
                    in_=out2[:, (b * 2 + oc) * S:(b * 2 + oc + 1) * S],
                    axis=AX.X, op=ALU.add)

        # ================== stage 3 =======================================
        bd3 = routing(2, r3w_sb, pool3, 2)
        cw3 = cwa.tile([P, BS * 2 * 1024], bf16, tag="cwa", name="cw3")
        cw3_v = cw3.rearrange("p (b ic g j) -> p ic g b j", b=BS, ic=2, g=64, j=16)
        combine(3, w3_sb, bd3, N_CHUNKS3 // 8,
                lambda bank: cw3_v[:, bank // 8, (bank % 8) * 8:(bank % 8) * 8 + 8])

        # conv3 + identity + bn3-bias + relu -> out
        for b in range(BS):
            ost = ostage.tile([P, 8 * S], f32, tag="ost", name="ost")
            for oc in range(8):
                ps = cpsum.tile([P, S], f32, tag="cps", name="ps_c3")
                nc.tensor.matmul(
                    ps, cw3[:, b * 2048 + 0 * 1024 + oc * P:
                            b * 2048 + 0 * 1024 + oc * P + P],
                    out2[:, b * 2 * S:b * 2 * S + S], start=True, stop=False)
                nc.tensor.matmul(
                    ps, cw3[:, b * 2048 + 1 * 1024 + oc * P:
                            b * 2048 + 1 * 1024 + oc * P + P],
                    out2[:, b * 2 * S + S:b * 2 * S + 2 * S], start=False, stop=False)
                nc.tensor.matmul(
                    ps, ident_sb,
                    x_sb[:, b * 8 * S + oc * S:b * 8 * S + (oc + 1) * S],
                    start=False, stop=True)
                nc.scalar.activation(
                    out=ost[:, oc * S:(oc + 1) * S], in_=ps, func=AF.Relu,
                    bias=beta_sb[:, 4 + oc:5 + oc], scale=1.0)
            nc.sync.dma_start(out=outd[:, b * 8 * S:(b + 1) * 8 * S], in_=ost)

    nc.finalize()
    return nc


# ----------------------------------------------------------------------------
# Entry point
# ----------------------------------------------------------------------------

def kernel(x, w1, w2, w3, r1_w, r1_b, r2_w, r2_b, r3_w, r3_b,
           bn1_g, bn1_b, bn1_m, bn1_v, bn2_g, bn2_b, bn2_m, bn2_v,
           bn3_g, bn3_b, bn3_m, bn3_v, _trace=False):
    global _nc_cache, last_exec_time_ns, last_trace_path
    from concourse.bass_utils import run_bass_kernel_spmd

    shared = _prep_weights(
        np.asarray(w1, np.float32), np.asarray(w2, np.float32),
        np.asarray(w3, np.float32),
        np.asarray(r1_w, np.float32), np.asarray(r1_b, np.float32),
        np.asarray(r2_w, np.float32), np.asarray(r2_b, np.float32),
        np.asarray(r3_w, np.float32), np.asarray(r3_b, np.float32),
        np.asarray(bn1_g, np.float32), np.asarray(bn1_b, np.float32),
        np.asarray(bn1_m, np.float32), np.asarray(bn1_v, np.float32),
        np.asarray(bn2_g, np.float32), np.asarray(bn2_b, np.float32),
        np.asarray(bn2_m, np.float32), np.asarray(bn2_v, np.float32),
        np.asarray(bn3_g, np.float32), np.asarray(bn3_b, np.float32),
        np.asarray(bn3_m, np.float32), np.asarray(bn3_v, np.float32))
    xs = _prep_x(np.asarray(x, np.float32))

    shared_map = {
        "w1r": shared["w1r"], "w2r": shared["w2r"], "w3r": shared["w3r"],
        "r1rep": shared["r1rep"].astype(np.float32),
        "r2rep": shared["r2rep"].astype(np.float32),
        "r3rep": shared["r3rep"].astype(np.float32),
        "rb_rep": shared["rb_rep"].astype(np.float32),
        "mask": shared["mask"], "beta": shared["beta"].astype(np.float32),
        "ident": shared["ident"],
    }
    in_maps = [dict(shared_map, x_bf=xs[c]) for c in range(NCORES)]

    if _nc_cache is None:
        _nc_cache = _build_nc()
    res = run_bass_kernel_spmd(_nc_cache, in_maps, core_ids=list(range(NCORES)),
                               trace=_trace)
    last_exec_time_ns = res.exec_time_ns
    last_trace_path = (res.instructions_and_trace or (None, None))[1]
    global last_res
    last_res = res

    out = np.empty((B, OUTP, H, H), np.float32)
    for c in range(NCORES):
        o = res.results[c]["out"]                       # [128, BS*8*196] f32
        out[c * BS:(c + 1) * BS] = (
            o.reshape(P, BS, 8, S).transpose(1, 2, 0, 3).reshape(BS, OUTP, H, H))
    return out
